# revision 3
# baseline (speedup 1.0000x reference)
"""NonLocalBlock (B=8, C=256, HW=64x64) Trainium2 kernel.

Data-parallel over batch: one sample per NeuronCore (8 cores).
Per core (all on-chip, the [N,N]=67MB attention matrix never touches HBM):

  x [C=256, N=4096] --DMA--> SBUF as fp32r (logit path + residual) and bf16
  theta = w_theta @ x + b_theta    [O=128, N]  fp32r   (PE fp32r, full speed)
  phi   = w_phi   @ x + b_phi      [O=128, N]  fp32r
  gt    = (w_g @ x)^T              [N, O] bf16 chunks + ones column
  per 512-wide n-tile (software-pipelined with the previous tile):
    S^T chunks [m=128, n=512] = phi_chunk^T . theta_tile   (PE fp32r, PSUM)
    P^T = exp(S^T)   (ScalarE, PSUM->SBUF bf16; logits <~55 so no max-sub)
    y[s] [128, 129] += P^T_chunk^T . [gt_chunk | ones]     (PE bf16)
        col 128 accumulates the softmax row-sum for free
    ynorm = y[:, :128] * (1/y[:,128]); yt = transpose(ynorm) + b_g
    z = w_out_tile^T . yt;  out = x + z*bn_scale + bn_shift;  DMA out

The attention inner loop is manually software-pipelined: for tile nt the
group loop emits [S^T(g), exp(g), mm2(g, s=0,1)] plus mm2(g, s=2,3) of tile
nt-1, so the PE always has bf16 matmul work while ScalarE streams exps.
PSUM: 4 banks S^T stack + a 4-slot ring shared by y accumulators /
transposes / output-projection tiles.
"""
import os
import sys

sys.path.insert(0, "/opt/trn_rl_repo")

import numpy as np
import ml_dtypes

import concourse.bass as bass
import concourse.bacc as bacc
import concourse.mybir as mybir
import concourse.tile as tile
from concourse.bass_utils import run_bass_kernel_spmd
from concourse.masks import make_identity

F32 = mybir.dt.float32
F32R = mybir.dt.float32r
BF16 = mybir.dt.bfloat16
ADD = mybir.AluOpType.add
MULT = mybir.AluOpType.mult
EXP = mybir.ActivationFunctionType.Exp

B, C, O, N = 8, 256, 128, 4096
NT = 512
N_TILES = N // NT           # 8
M_CHUNKS = N // 128         # 32
GRPS = M_CHUNKS // 4        # 8 groups of 4 m-chunks per exp ACTIVATE
BN_EPS = 1e-5


def build_nc():
    nc = bacc.Bacc()

    xin = nc.dram_tensor("xin", [C, N], F32R, kind="ExternalInput")
    xbf = nc.dram_tensor("xbf", [C, N], BF16, kind="ExternalInput")
    wth = nc.dram_tensor("wth", [C, O], F32R, kind="ExternalInput")    # w_theta.T
    wph = nc.dram_tensor("wph", [C, O], F32R, kind="ExternalInput")    # w_phi.T
    wg = nc.dram_tensor("wg", [C, O], BF16, kind="ExternalInput")      # w_g.T
    wout = nc.dram_tensor("wout", [O, C], BF16, kind="ExternalInput")  # w_out.T
    bth = nc.dram_tensor("bth", [O, 1], F32, kind="ExternalInput")
    bph = nc.dram_tensor("bph", [O, 1], F32, kind="ExternalInput")
    bg = nc.dram_tensor("bg", [O, 1], F32, kind="ExternalInput")
    bnscale = nc.dram_tensor("bnscale", [128, 2], F32, kind="ExternalInput")
    bnshift = nc.dram_tensor("bnshift", [128, 2], F32, kind="ExternalInput")
    out = nc.dram_tensor("out", [C, N], F32, kind="ExternalOutput")

    with tile.TileContext(nc) as tc:
        with tc.tile_pool(name="const", bufs=1) as const, \
             tc.tile_pool(name="xpool", bufs=1) as xpool, \
             tc.tile_pool(name="proj", bufs=1) as proj, \
             tc.tile_pool(name="yt_pool", bufs=3) as yt_pool, \
             tc.tile_pool(name="small", bufs=6) as small, \
             tc.tile_pool(name="ostage", bufs=4) as ostage:

            # ---- constants ----
            wth_r = const.tile([128, 2 * O], F32R)
            wph_r = const.tile([128, 2 * O], F32R)
            wg_sb = const.tile([128, 2 * O], BF16)
            for k in range(2):
                csl = slice(k * 128, (k + 1) * 128)
                osl = slice(k * O, (k + 1) * O)
                nc.sync.dma_start(wth_r[:, osl], wth[csl, :])
                nc.sync.dma_start(wph_r[:, osl], wph[csl, :])
                nc.sync.dma_start(wg_sb[:, osl], wg[csl, :])
            wout_sb = const.tile([O, C], BF16)
            nc.sync.dma_start(wout_sb[:], wout[:])
            bth_sb = const.tile([O, 1], F32)
            bph_sb = const.tile([O, 1], F32)
            bg_sb = const.tile([O, 1], F32)
            bnscale_sb = const.tile([128, 2], F32)
            bnshift_sb = const.tile([128, 2], F32)
            nc.sync.dma_start(bth_sb[:], bth[:])
            nc.sync.dma_start(bph_sb[:], bph[:])
            nc.sync.dma_start(bg_sb[:], bg[:])
            nc.sync.dma_start(bnscale_sb[:], bnscale[:])
            nc.sync.dma_start(bnshift_sb[:], bnshift[:])
            ident = const.tile([128, 128], BF16)
            make_identity(nc, ident)

            # ---- x loads (direct: fp32r for logit path + residual, bf16 for g) ----
            x_r = [xpool.tile([128, N], F32R, name=f"x{i}_r") for i in range(2)]
            x_bf = [xpool.tile([128, N], BF16, name=f"x{i}_bf") for i in range(2)]
            for i in range(2):
                rsl = slice(i * 128, (i + 1) * 128)
                for h in range(4):
                    hsl = slice(h * (N // 4), (h + 1) * (N // 4))
                    nc.sync.dma_start(x_r[i][:, hsl], xin[rsl, hsl])
                    nc.sync.dma_start(x_bf[i][:, hsl], xbf[rsl, hsl])

            theta_r = proj.tile([O, N], F32R)
            phi_r = proj.tile([O, N], F32R)
            gt_sb = proj.tile([128, M_CHUNKS * (O + 1)], BF16)

            with tc.tile_pool(name="thph_ps", bufs=2, space="PSUM") as thph_ps, \
                 tc.tile_pool(name="gt_ps", bufs=2, space="PSUM") as gt_ps:
                # phi first (the S^T pass reads all of phi), then theta, then gt
                for (dst, w_r, b_sb) in ((phi_r, wph_r, bph_sb), (theta_r, wth_r, bth_sb)):
                    for part in range(4):
                        pp = thph_ps.tile([128, 1024], F32)
                        for j in range(2):
                            cb = part * 1024 + j * 512
                            for k in range(2):
                                nc.tensor.matmul(
                                    pp[:, j * 512:(j + 1) * 512],
                                    w_r[:, k * O:(k + 1) * O],
                                    x_r[k][:, cb:cb + 512],
                                    start=(k == 0), stop=(k == 1),
                                )
                        psl = slice(part * 1024, (part + 1) * 1024)
                        nc.vector.tensor_scalar_add(dst[:, psl], pp[:], b_sb[:])

                for c in range(M_CHUNKS):
                    gp = gt_ps.tile([128, O], F32)
                    for k in range(2):
                        nc.tensor.matmul(
                            gp[:],
                            x_bf[k][:, c * 128:(c + 1) * 128],
                            wg_sb[:, k * O:(k + 1) * O],
                            start=(k == 0), stop=(k == 1),
                        )
                    base = c * (O + 1)
                    nc.vector.tensor_copy(gt_sb[:, base:base + O], gp[:])
                    nc.gpsimd.memset(gt_sb[:, base + O:base + O + 1], 1.0)

            # ---- software-pipelined attention ----
            with tc.tile_pool(name="pt_pool", bufs=12) as pt_pool, \
                 tc.tile_pool(name="st_ps", bufs=1, space="PSUM") as st_ps, \
                 tc.tile_pool(name="ya", bufs=4, space="PSUM") as ya:

                def tail_half(y, yt_tile, col):
                    recip = small.tile([128, 1], F32)
                    nc.vector.reciprocal(recip[:], y[:, O:O + 1])
                    ynorm = small.tile([128, O], BF16)
                    nc.vector.tensor_scalar_mul(ynorm[:], y[:, 0:O], recip[:])
                    tr = ya.tile([128, 128], BF16, name="tr", tag="ya")
                    nc.tensor.transpose(tr[:], ynorm[:], ident[:])
                    nc.vector.tensor_scalar_add(yt_tile[:, col:col + 128], tr[:], bg_sb[:])

                def mm2(y, pt, s, g):
                    for k in range(4):
                        c = 4 * g + k
                        nc.tensor.matmul(
                            y[:],
                            pt[:, k * NT + s * 128:k * NT + (s + 1) * 128],
                            gt_sb[:, c * (O + 1):(c + 1) * (O + 1)],
                            start=(c == 0), stop=(c == M_CHUNKS - 1),
                        )

                pts_prev, yt_prev, y_prev = None, None, None
                for it in range(N_TILES + 1):
                    nt, pv = (it if it < N_TILES else None), (it - 1 if it > 0 else None)
                    if nt is not None:
                        ntsl = slice(nt * NT, (nt + 1) * NT)
                        y0 = ya.tile([128, O + 1], F32, name="y0", tag="ya")
                        y1 = ya.tile([128, O + 1], F32, name="y1", tag="ya")
                    if pv is not None:
                        y2 = ya.tile([128, O + 1], F32, name="y2", tag="ya")
                        y3 = ya.tile([128, O + 1], F32, name="y3", tag="ya")
                    pts_cur = []
                    for g in range(GRPS):
                        if nt is not None:
                            st = st_ps.tile([128, 4 * NT], F32)
                            for k in range(4):
                                c = 4 * g + k
                                nc.tensor.matmul(
                                    st[:, k * NT:(k + 1) * NT],
                                    phi_r[:, c * 128:(c + 1) * 128],
                                    theta_r[:, ntsl],
                                    start=True, stop=True,
                                )
                            pt = pt_pool.tile([128, 4 * NT], BF16)
                            nc.scalar.activation(pt[:], st[:], EXP)
                            pts_cur.append(pt)
                            mm2(y0, pt, 0, g)
                            mm2(y1, pt, 1, g)
                        if pv is not None:
                            mm2(y2, pts_prev[g], 2, g)
                            mm2(y3, pts_prev[g], 3, g)
                    if nt is not None:
                        yt_cur = yt_pool.tile([O, NT], BF16)
                        tail_half(y0, yt_cur, 0)
                        tail_half(y1, yt_cur, 128)
                    if pv is not None:
                        tail_half(y2, yt_prev, 256)
                        tail_half(y3, yt_prev, 384)
                        pvsl = slice(pv * NT, (pv + 1) * NT)
                        for ct in range(2):
                            o_ps = ya.tile([128, NT], F32, name="o_ps", tag="ya")
                            nc.tensor.matmul(
                                o_ps[:],
                                wout_sb[:, ct * 128:(ct + 1) * 128],
                                yt_prev[:],
                                start=True, stop=True,
                            )
                            obn = ostage.tile([128, NT], F32)
                            nc.vector.tensor_scalar(
                                obn[:], o_ps[:],
                                bnscale_sb[:, ct:ct + 1], bnshift_sb[:, ct:ct + 1],
                                op0=MULT, op1=ADD,
                            )
                            ores = ostage.tile([128, NT], F32, name="ores")
                            nc.vector.tensor_tensor(
                                ores[:], obn[:], x_r[ct][:, pvsl].bitcast(F32), op=ADD
                            )
                            nc.sync.dma_start(out[ct * 128:(ct + 1) * 128, pvsl], ores[:])
                    if nt is not None:
                        pts_prev, yt_prev = pts_cur, yt_cur

    nc.finalize()
    return nc


_NC_CACHE = None


def _get_nc():
    global _NC_CACHE
    if _NC_CACHE is None:
        _NC_CACHE = build_nc()
    return _NC_CACHE


def _prepare_in_maps(inputs):
    x = np.ascontiguousarray(np.asarray(inputs["x"], dtype=np.float32)).reshape(B, C, N)
    xbf = x.astype(ml_dtypes.bfloat16)
    wth = np.ascontiguousarray(np.asarray(inputs["w_theta"], np.float32).T)
    wph = np.ascontiguousarray(np.asarray(inputs["w_phi"], np.float32).T)
    wg = np.ascontiguousarray(np.asarray(inputs["w_g"], np.float32).T).astype(ml_dtypes.bfloat16)
    wout = np.ascontiguousarray(np.asarray(inputs["w_out"], np.float32).T).astype(ml_dtypes.bfloat16)
    bth = np.asarray(inputs["b_theta"], np.float32).reshape(O, 1)
    bph = np.asarray(inputs["b_phi"], np.float32).reshape(O, 1)
    bg = np.asarray(inputs["b_g"], np.float32).reshape(O, 1)
    inv = np.asarray(inputs["bn_gamma"], np.float32) / np.sqrt(
        np.asarray(inputs["bn_var"], np.float32) + BN_EPS)
    shift = (np.asarray(inputs["b_out"], np.float32) * inv
             + np.asarray(inputs["bn_beta"], np.float32)
             - np.asarray(inputs["bn_mean"], np.float32) * inv)
    bnscale = np.ascontiguousarray(inv.reshape(2, 128).T)
    bnshift = np.ascontiguousarray(shift.reshape(2, 128).T)

    shared = dict(wth=wth, wph=wph, wg=wg, wout=wout, bth=bth, bph=bph,
                  bg=bg, bnscale=bnscale, bnshift=bnshift)
    return [dict(shared, xin=np.ascontiguousarray(x[b]),
                 xbf=np.ascontiguousarray(xbf[b])) for b in range(B)]


def _install_ntff_shim():
    """This image's antenv lacks axon_hooks; provide it from trn_boot's
    ctypes implementation so trace=True can capture NTFF profiles."""
    import types
    try:
        import antenv.axon_hooks  # noqa: F401
        return
    except ImportError:
        pass
    if "/root/.axon_site" not in sys.path:
        sys.path.insert(0, "/root/.axon_site")
    from trn_agent_boot.trn_boot import _ntff_profile_via_ctypes
    hook = _ntff_profile_via_ctypes("/opt/axon/libaxon_pjrt.so")
    m = types.ModuleType("antenv.axon_hooks")
    m.get_axon_ntff_profile_hook = lambda: hook
    m.set_axon_ntff_profile_hook = lambda h: None
    sys.modules["antenv.axon_hooks"] = m


def run(inputs, trace=False):
    if trace:
        _install_ntff_shim()
    nc = _get_nc()
    in_maps = _prepare_in_maps(inputs)
    res = run_bass_kernel_spmd(nc, in_maps, list(range(B)), trace=trace)
    outs = np.stack([res.results[b]["out"] for b in range(B)])
    return outs.reshape(B, C, 64, 64), res


def kernel(**inputs) -> np.ndarray:
    out, _ = run(inputs)
    return out


if __name__ == "__main__":
    # quick CoreSim check of one core
    from concourse import bass_interp
    rng = np.random.default_rng(0)
    fake = {
        "x": rng.standard_normal((B, C, 64, 64)).astype(np.float32),
        "w_theta": (rng.standard_normal((O, C)) * 0.05).astype(np.float32),
        "b_theta": (rng.standard_normal(O) * 0.05).astype(np.float32),
        "w_phi": (rng.standard_normal((O, C)) * 0.05).astype(np.float32),
        "b_phi": (rng.standard_normal(O) * 0.05).astype(np.float32),
        "w_g": (rng.standard_normal((O, C)) * 0.05).astype(np.float32),
        "b_g": (rng.standard_normal(O) * 0.05).astype(np.float32),
        "w_out": (rng.standard_normal((C, O)) * 0.05).astype(np.float32),
        "b_out": (rng.standard_normal(C) * 0.05).astype(np.float32),
        "bn_gamma": rng.standard_normal(C).astype(np.float32),
        "bn_beta": rng.standard_normal(C).astype(np.float32),
        "bn_mean": rng.standard_normal(C).astype(np.float32),
        "bn_var": rng.uniform(0.5, 1.5, C).astype(np.float32),
    }
    nc = _get_nc()
    in_maps = _prepare_in_maps(fake)
    sim = bass_interp.CoreSim(nc)
    for k, v in in_maps[0].items():
        sim.tensor(k)[:] = v
    sim.simulate()
    got = np.asarray(sim.tensor("out"))

    x0 = fake["x"][0].reshape(C, N)
    th = fake["w_theta"] @ x0 + fake["b_theta"][:, None]
    ph = fake["w_phi"] @ x0 + fake["b_phi"][:, None]
    gg = fake["w_g"] @ x0 + fake["b_g"][:, None]
    s = th.T @ ph
    p = np.exp(s - s.max(1, keepdims=True))
    a = p / p.sum(1, keepdims=True)
    yy = a @ gg.T
    wy = fake["w_out"] @ yy.T + fake["b_out"][:, None]
    inv = fake["bn_gamma"] / np.sqrt(fake["bn_var"] + BN_EPS)
    bn = wy * inv[:, None] + (fake["bn_beta"] - fake["bn_mean"] * inv)[:, None]
    want = x0 + bn
    err = np.abs(got - want).max()
    print("CoreSim absmax err:", err, "rel:", err / np.abs(want).max())


# revision 7
# speedup vs baseline: 1.4506x; 1.4506x over previous
"""NonLocalBlock (B=8, C=256, HW=64x64) Trainium2 kernel.

Data-parallel over batch: one sample per NeuronCore (8 cores).
Per core (everything on-chip; the [N,N]=67MB attention matrix never
touches HBM):

  x [C=256, N=4096] fp16 --DMA--> SBUF (serves logit path, g path, residual)
  theta = w_theta @ x + b_theta    [O=128, N]  fp16   (PE fp16, FWL)
  phi   = w_phi   @ x + b_phi      [O=128, N]  fp16
  gt    = (w_g @ x)^T              [N, O] bf16 chunks + ones column
  per 512-wide n-tile, software-pipelined with the previous tile:
    S^T chunks [m=128, n=512] = phi_chunk^T . theta_tile  (PE fp16 -> PSUM)
    P^T = exp(S^T)  (ScalarE, PSUM->SBUF bf16; logits <~55, no max-subtract;
                     bf16 holds exp range up to 3e38)
    y[s] [128, 129] += P^T_chunk^T . [gt_chunk | ones]    (PE bf16)
        col 128 accumulates the softmax row-sum for free
    ynorm = y[:, :128] * (1/y[:,128])    (DVE)
    yt[o, n-sub] = DMA-xbar-transpose(ynorm)              (no PSUM, no PE)
    z = w_out_tile^T . yt;  out = x + z*bn_scale + bn_shift';  DMA out
       (b_g is folded into bn_shift' on the host: w_out @ b_g is constant)

PSUM (8 banks): S^T groups of 3 chunks [128,1536] double-buffered (6 banks)
+ a 2-slot ring where two [128,129] y-accumulators pack into one bank (the
second starts with start=False onto the bank cleared by the first's
start=True) and the output-projection tiles reuse the same slots.

The group loop interleaves, per group g: S^T(g) matmuls, exp(g), mm2 of
group g-1 (s=0,1) for this tile, and mm2 of group g (s=2,3) for the
PREVIOUS tile — so the PE always has ready bf16 matmul work while ScalarE
streams exps, and exp(g+1) never waits on PE (S^T double-buffered).
"""
import os
import sys

sys.path.insert(0, "/opt/trn_rl_repo")

import numpy as np
import ml_dtypes

import concourse.bass as bass
import concourse.bacc as bacc
import concourse.mybir as mybir
import concourse.tile as tile
from concourse.tile import add_dep_helper
from concourse.bass_utils import run_bass_kernel_spmd

F32 = mybir.dt.float32
F16 = mybir.dt.float16
BF16 = mybir.dt.bfloat16
ADD = mybir.AluOpType.add
MULT = mybir.AluOpType.mult
EXP = mybir.ActivationFunctionType.Exp

B, C, O, N = 8, 256, 128, 4096
NT = 512
N_TILES = N // NT            # 8
M_CHUNKS = N // 128          # 32
# S^T groups: chunks per exp ACTIVATE (3 banks => double-buffered in 6)
GROUPS = [(c, min(c + 3, M_CHUNKS)) for c in range(0, M_CHUNKS, 3)]  # 11 groups
BN_EPS = 1e-5


def build_nc():
    nc = bacc.Bacc()

    xin = nc.dram_tensor("xin", [C, N], F16, kind="ExternalInput")
    wth = nc.dram_tensor("wth", [C, O], F16, kind="ExternalInput")     # w_theta.T
    wph = nc.dram_tensor("wph", [C, O], F16, kind="ExternalInput")     # w_phi.T
    wg = nc.dram_tensor("wg", [C, O], F16, kind="ExternalInput")       # w_g.T
    wout = nc.dram_tensor("wout", [O, C], BF16, kind="ExternalInput")  # w_out.T
    bth = nc.dram_tensor("bth", [O, 1], F32, kind="ExternalInput")
    bph = nc.dram_tensor("bph", [O, 1], F32, kind="ExternalInput")
    bnscale = nc.dram_tensor("bnscale", [128, 2], F32, kind="ExternalInput")
    bnshift = nc.dram_tensor("bnshift", [128, 2], F32, kind="ExternalInput")
    out = nc.dram_tensor("out", [C, N], F32, kind="ExternalOutput")

    with tile.TileContext(nc) as tc:
        with tc.tile_pool(name="const", bufs=1) as const, \
             tc.tile_pool(name="xpool", bufs=1) as xpool, \
             tc.tile_pool(name="proj", bufs=1) as proj, \
             tc.tile_pool(name="yt_pool", bufs=3) as yt_pool, \
             tc.tile_pool(name="small", bufs=6) as small, \
             tc.tile_pool(name="ostage", bufs=4) as ostage:

            # ---- constants ----
            wth_sb = const.tile([128, 2 * O], F16)
            wph_sb = const.tile([128, 2 * O], F16)
            wg_sb = const.tile([128, 2 * O], F16)
            for k in range(2):
                csl = slice(k * 128, (k + 1) * 128)
                osl = slice(k * O, (k + 1) * O)
                nc.sync.dma_start(wth_sb[:, osl], wth[csl, :])
                nc.sync.dma_start(wph_sb[:, osl], wph[csl, :])
                nc.sync.dma_start(wg_sb[:, osl], wg[csl, :])
            wout_sb = const.tile([O, C], BF16)
            nc.sync.dma_start(wout_sb[:], wout[:])
            bth_sb = const.tile([O, 1], F32)
            bph_sb = const.tile([O, 1], F32)
            bnscale_sb = const.tile([128, 2], F32)
            bnshift_sb = const.tile([128, 2], F32)
            nc.sync.dma_start(bth_sb[:], bth[:])
            nc.sync.dma_start(bph_sb[:], bph[:])
            nc.sync.dma_start(bnscale_sb[:], bnscale[:])
            nc.sync.dma_start(bnshift_sb[:], bnshift[:])

            # ---- x load (fp16, one copy serves everything) ----
            x_h = [xpool.tile([128, N], F16, name=f"x{i}_h") for i in range(2)]
            for i in range(2):
                rsl = slice(i * 128, (i + 1) * 128)
                for h in range(4):
                    hsl = slice(h * (N // 4), (h + 1) * (N // 4))
                    nc.sync.dma_start(x_h[i][:, hsl], xin[rsl, hsl])

            theta_h = proj.tile([O, N], F16)
            phi_h = proj.tile([O, N], F16)
            gt_sb = proj.tile([128, M_CHUNKS * (O + 1)], BF16)

            with tc.tile_pool(name="thph_ps", bufs=2, space="PSUM") as thph_ps, \
                 tc.tile_pool(name="gt_ps", bufs=2, space="PSUM") as gt_ps:
                # phi first (the S^T pass needs all of phi), then theta, then gt
                for (dst, w_sb, b_sb) in ((phi_h, wph_sb, bph_sb), (theta_h, wth_sb, bth_sb)):
                    for part in range(4):
                        pp = thph_ps.tile([128, 1024], F32)
                        for j in range(2):
                            cb = part * 1024 + j * 512
                            for k in range(2):
                                nc.tensor.matmul(
                                    pp[:, j * 512:(j + 1) * 512],
                                    w_sb[:, k * O:(k + 1) * O],
                                    x_h[k][:, cb:cb + 512],
                                    start=(k == 0), stop=(k == 1),
                                )
                        psl = slice(part * 1024, (part + 1) * 1024)
                        nc.vector.tensor_scalar_add(dst[:, psl], pp[:], b_sb[:])

                for c in range(M_CHUNKS):
                    gp = gt_ps.tile([128, O], F32)
                    for k in range(2):
                        nc.tensor.matmul(
                            gp[:],
                            x_h[k][:, c * 128:(c + 1) * 128],
                            wg_sb[:, k * O:(k + 1) * O],
                            start=(k == 0), stop=(k == 1),
                        )
                    base = c * (O + 1)
                    nc.vector.tensor_copy(gt_sb[:, base:base + O], gp[:])
                    nc.gpsimd.memset(gt_sb[:, base + O:base + O + 1], 1.0)

            # ---- software-pipelined attention ----
            with tc.tile_pool(name="pt_pool", bufs=16) as pt_pool, \
                 tc.tile_pool(name="st_ps", bufs=2, space="PSUM") as st_ps, \
                 tc.tile_pool(name="ya", bufs=2, space="PSUM") as ya:

                def mm2(y, pt, s, c0, c1, pt_c0):
                    # y accumulator slice gets chunks [c0, c1) of P^T tile pt.
                    # Two accumulators share one PSUM bank: only the first
                    # (s even) opens the group (start=True clears the whole
                    # bank); the second writes start=False onto cleared bits.
                    first = None
                    for c in range(c0, c1):
                        i = nc.tensor.matmul(
                            y[:],
                            pt[:, (c - pt_c0) * NT + s * 128:(c - pt_c0) * NT + (s + 1) * 128],
                            gt_sb[:, c * (O + 1):(c + 1) * (O + 1)],
                            start=(c == 0 and s % 2 == 0),
                            stop=(c == M_CHUNKS - 1),
                            skip_group_check=True,
                        )
                        if first is None:
                            first = i
                    return first

                def norm_transpose(y, yt_tile, col):
                    recip = small.tile([128, 1], F32)
                    nc.vector.reciprocal(recip[:], y[:, O:O + 1])
                    ynorm = small.tile([128, O], BF16)
                    nc.vector.tensor_scalar_mul(ynorm[:], y[:, 0:O], recip[:])
                    nc.sync.dma_start_transpose(yt_tile[:, col:col + 128], ynorm[:])

                pts_prev, yt_prev, y23_prev = None, None, None
                for it in range(N_TILES + 1):
                    nt = it if it < N_TILES else None
                    pv = it - 1 if it > 0 else None
                    if nt is not None:
                        ntsl = slice(nt * NT, (nt + 1) * NT)
                        y01 = ya.tile([128, 260], F32, name="y01", tag="ya")
                    if pv is not None:
                        y23 = ya.tile([128, 260], F32, name="y23", tag="ya")
                    pts_cur = []
                    n_grps = len(GROUPS)
                    for g in range(n_grps):
                        if nt is not None:
                            c0, c1 = GROUPS[g]
                            w = (c1 - c0) * NT
                            st = st_ps.tile([128, 3 * NT], F32)
                            for c in range(c0, c1):
                                nc.tensor.matmul(
                                    st[:, (c - c0) * NT:(c - c0 + 1) * NT],
                                    phi_h[:, c * 128:(c + 1) * 128],
                                    theta_h[:, ntsl],
                                    start=True, stop=True,
                                )
                            pt = pt_pool.tile([128, 3 * NT], BF16)
                            nc.scalar.activation(pt[:, :w], st[:, :w], EXP)
                            pts_cur.append(pt)
                            if g > 0:
                                pc0, pc1 = GROUPS[g - 1]
                                ia = mm2(y01[:, 0:O + 1], pts_cur[g - 1], 0, pc0, pc1, pc0)
                                ib = mm2(y01[:, 130:259], pts_cur[g - 1], 1, pc0, pc1, pc0)
                                if pc0 == 0:
                                    add_dep_helper(ib.ins, ia.ins, sync=False,
                                                   reason="bank-pack: clear before first write")
                        if pv is not None:
                            c0, c1 = GROUPS[g]
                            ia = mm2(y23[:, 0:O + 1], pts_prev[g], 2, c0, c1, c0)
                            ib = mm2(y23[:, 130:259], pts_prev[g], 3, c0, c1, c0)
                            if c0 == 0:
                                add_dep_helper(ib.ins, ia.ins, sync=False,
                                               reason="bank-pack: clear before first write")
                    if nt is not None:
                        c0, c1 = GROUPS[-1]
                        mm2(y01[:, 0:O + 1], pts_cur[-1], 0, c0, c1, c0)
                        mm2(y01[:, 130:259], pts_cur[-1], 1, c0, c1, c0)
                        yt_cur = yt_pool.tile([O, NT], BF16)
                        norm_transpose(y01[:, 0:O + 1], yt_cur, 0)
                        norm_transpose(y01[:, 130:259], yt_cur, 128)
                    if pv is not None:
                        norm_transpose(y23[:, 0:O + 1], yt_prev, 256)
                        norm_transpose(y23[:, 130:259], yt_prev, 384)
                        pvsl = slice(pv * NT, (pv + 1) * NT)
                        for ct in range(2):
                            o_ps = ya.tile([128, NT], F32, name="o_ps", tag="ya")
                            nc.tensor.matmul(
                                o_ps[:],
                                wout_sb[:, ct * 128:(ct + 1) * 128],
                                yt_prev[:],
                                start=True, stop=True,
                            )
                            obn = ostage.tile([128, NT], F32)
                            nc.vector.tensor_scalar(
                                obn[:], o_ps[:],
                                bnscale_sb[:, ct:ct + 1], bnshift_sb[:, ct:ct + 1],
                                op0=MULT, op1=ADD,
                            )
                            ores = ostage.tile([128, NT], F32, name="ores")
                            nc.vector.tensor_tensor(
                                ores[:], obn[:], x_h[ct][:, pvsl], op=ADD
                            )
                            nc.sync.dma_start(out[ct * 128:(ct + 1) * 128, pvsl], ores[:])
                    if nt is not None:
                        pts_prev, yt_prev = pts_cur, yt_cur

    nc.finalize()
    return nc


_NC_CACHE = None


def _get_nc():
    global _NC_CACHE
    if _NC_CACHE is None:
        _NC_CACHE = build_nc()
    return _NC_CACHE


def _prepare_in_maps(inputs):
    x = np.ascontiguousarray(np.asarray(inputs["x"], dtype=np.float32)).reshape(B, C, N)
    xh = x.astype(np.float16)
    wth = np.ascontiguousarray(np.asarray(inputs["w_theta"], np.float32).T).astype(np.float16)
    wph = np.ascontiguousarray(np.asarray(inputs["w_phi"], np.float32).T).astype(np.float16)
    wg = np.ascontiguousarray(np.asarray(inputs["w_g"], np.float32).T).astype(np.float16)
    w_out = np.asarray(inputs["w_out"], np.float32)
    wout = np.ascontiguousarray(w_out.T).astype(ml_dtypes.bfloat16)
    bth = np.asarray(inputs["b_theta"], np.float32).reshape(O, 1)
    bph = np.asarray(inputs["b_phi"], np.float32).reshape(O, 1)
    inv = np.asarray(inputs["bn_gamma"], np.float32) / np.sqrt(
        np.asarray(inputs["bn_var"], np.float32) + BN_EPS)
    shift = (np.asarray(inputs["b_out"], np.float32) * inv
             + np.asarray(inputs["bn_beta"], np.float32)
             - np.asarray(inputs["bn_mean"], np.float32) * inv)
    # fold the g-branch bias through the output projection: softmax rows sum
    # to 1, so attn @ (g + b_g) = attn @ g + b_g, and w_out @ b_g is constant
    wob = wout.astype(np.float32) .T @ np.asarray(inputs["b_g"], np.float32)
    shift = shift + inv * wob
    bnscale = np.ascontiguousarray(inv.reshape(2, 128).T)
    bnshift = np.ascontiguousarray(shift.reshape(2, 128).T)

    shared = dict(wth=wth, wph=wph, wg=wg, wout=wout, bth=bth, bph=bph,
                  bnscale=bnscale, bnshift=bnshift)
    return [dict(shared, xin=np.ascontiguousarray(xh[b])) for b in range(B)]


def _install_ntff_shim():
    """This image's antenv lacks axon_hooks; provide it from trn_boot's
    ctypes implementation so trace=True can capture NTFF profiles."""
    import types
    try:
        import antenv.axon_hooks  # noqa: F401
        return
    except ImportError:
        pass
    if "/root/.axon_site" not in sys.path:
        sys.path.insert(0, "/root/.axon_site")
    from trn_agent_boot.trn_boot import _ntff_profile_via_ctypes
    hook = _ntff_profile_via_ctypes("/opt/axon/libaxon_pjrt.so")
    m = types.ModuleType("antenv.axon_hooks")
    m.get_axon_ntff_profile_hook = lambda: hook
    m.set_axon_ntff_profile_hook = lambda h: None
    sys.modules["antenv.axon_hooks"] = m


def run(inputs, trace=False):
    if trace:
        _install_ntff_shim()
    nc = _get_nc()
    in_maps = _prepare_in_maps(inputs)
    res = run_bass_kernel_spmd(nc, in_maps, list(range(B)), trace=trace)
    outs = np.stack([res.results[b]["out"] for b in range(B)])
    return outs.reshape(B, C, 64, 64), res


def kernel(**inputs) -> np.ndarray:
    out, _ = run(inputs)
    return out


if __name__ == "__main__":
    # quick CoreSim check of one core
    from concourse import bass_interp
    rng = np.random.default_rng(0)
    fake = {
        "x": rng.standard_normal((B, C, 64, 64)).astype(np.float32),
        "w_theta": (rng.standard_normal((O, C)) * 0.05).astype(np.float32),
        "b_theta": (rng.standard_normal(O) * 0.05).astype(np.float32),
        "w_phi": (rng.standard_normal((O, C)) * 0.05).astype(np.float32),
        "b_phi": (rng.standard_normal(O) * 0.05).astype(np.float32),
        "w_g": (rng.standard_normal((O, C)) * 0.05).astype(np.float32),
        "b_g": (rng.standard_normal(O) * 0.05).astype(np.float32),
        "w_out": (rng.standard_normal((C, O)) * 0.05).astype(np.float32),
        "b_out": (rng.standard_normal(C) * 0.05).astype(np.float32),
        "bn_gamma": rng.standard_normal(C).astype(np.float32),
        "bn_beta": rng.standard_normal(C).astype(np.float32),
        "bn_mean": rng.standard_normal(C).astype(np.float32),
        "bn_var": rng.uniform(0.5, 1.5, C).astype(np.float32),
    }
    nc = _get_nc()
    in_maps = _prepare_in_maps(fake)
    sim = bass_interp.CoreSim(nc)
    for k, v in in_maps[0].items():
        sim.tensor(k)[:] = v
    sim.simulate()
    got = np.asarray(sim.tensor("out"))

    x0 = fake["x"][0].reshape(C, N)
    th = fake["w_theta"] @ x0 + fake["b_theta"][:, None]
    ph = fake["w_phi"] @ x0 + fake["b_phi"][:, None]
    gg = fake["w_g"] @ x0 + fake["b_g"][:, None]
    s = th.T @ ph
    p = np.exp(s - s.max(1, keepdims=True))
    a = p / p.sum(1, keepdims=True)
    yy = a @ gg.T
    wy = fake["w_out"] @ yy.T + fake["b_out"][:, None]
    inv = fake["bn_gamma"] / np.sqrt(fake["bn_var"] + BN_EPS)
    bn = wy * inv[:, None] + (fake["bn_beta"] - fake["bn_mean"] * inv)[:, None]
    want = x0 + bn
    err = np.abs(got - want).max()
    print("CoreSim absmax err:", err, "rel:", err / np.abs(want).max())


# revision 9
# speedup vs baseline: 1.5130x; 1.0430x over previous
"""NonLocalBlock (B=8, C=256, HW=64x64) Trainium2 kernel.

Data-parallel over batch: one sample per NeuronCore (8 cores).
Per core (everything on-chip; the [N,N]=67MB attention matrix never
touches HBM):

  x [C=256, N=4096] fp16 --DMA--> SBUF (serves logit path, g path, residual)
  theta = w_theta @ x + b_theta    [O=128, N]  fp16   (PE fp16, FWL)
  phi   = w_phi   @ x + b_phi      [O=128, N]  fp16
  gt    = (w_g @ x)^T              [N, O] bf16 chunks + ones column
  per 512-wide n-tile, software-pipelined with the previous tile:
    S^T chunks [m=128, n=512] = phi_chunk^T . theta_tile  (PE fp16 -> PSUM)
    P^T = exp(S^T)  (ScalarE, PSUM->SBUF bf16; logits <~55, no max-subtract;
                     bf16 holds exp range up to 3e38)
    y[s] [128, 129] += P^T_chunk^T . [gt_chunk | ones]    (PE bf16)
        col 128 accumulates the softmax row-sum for free
    ynorm = y[:, :128] * (1/y[:,128])    (DVE)
    yt[o, n-sub] = DMA-xbar-transpose(ynorm)              (no PSUM, no PE)
    z = w_out_tile^T . yt;  out = x + z*bn_scale + bn_shift';  DMA out
       (b_g is folded into bn_shift' on the host: w_out @ b_g is constant)

PSUM (8 banks): S^T groups of 3 chunks [128,1536] double-buffered (6 banks)
+ a 2-slot ring where two [128,129] y-accumulators pack into one bank (the
second starts with start=False onto the bank cleared by the first's
start=True) and the output-projection tiles reuse the same slots.

The group loop interleaves, per group g: S^T(g) matmuls, exp(g), mm2 of
group g-1 (s=0,1) for this tile, and mm2 of group g (s=2,3) for the
PREVIOUS tile — so the PE always has ready bf16 matmul work while ScalarE
streams exps, and exp(g+1) never waits on PE (S^T double-buffered).
"""
import os
import sys

sys.path.insert(0, "/opt/trn_rl_repo")

import numpy as np
import ml_dtypes

import concourse.bass as bass
import concourse.bacc as bacc
import concourse.mybir as mybir
import concourse.tile as tile
from concourse.tile import add_dep_helper
from concourse.bass_utils import run_bass_kernel_spmd

F32 = mybir.dt.float32
F16 = mybir.dt.float16
BF16 = mybir.dt.bfloat16
ADD = mybir.AluOpType.add
MULT = mybir.AluOpType.mult
EXP = mybir.ActivationFunctionType.Exp

B, C, O, N = 8, 256, 128, 4096
NT = 512
N_TILES = N // NT            # 8
M_CHUNKS = N // 128          # 32
# S^T groups: chunks per exp ACTIVATE (3 banks => double-buffered in 6)
GROUPS = [(c, min(c + 3, M_CHUNKS)) for c in range(0, M_CHUNKS, 3)]  # 11 groups
BN_EPS = 1e-5


def build_nc():
    nc = bacc.Bacc()

    xin = nc.dram_tensor("xin", [C, N], F16, kind="ExternalInput")
    wth = nc.dram_tensor("wth", [C, O], F16, kind="ExternalInput")     # w_theta.T
    wph = nc.dram_tensor("wph", [C, O], F16, kind="ExternalInput")     # w_phi.T
    wg = nc.dram_tensor("wg", [C, O], F16, kind="ExternalInput")       # w_g.T
    wout = nc.dram_tensor("wout", [O, C], BF16, kind="ExternalInput")  # w_out.T
    bth = nc.dram_tensor("bth", [O, 1], F32, kind="ExternalInput")
    bph = nc.dram_tensor("bph", [O, 1], F32, kind="ExternalInput")
    bnscale = nc.dram_tensor("bnscale", [128, 2], F32, kind="ExternalInput")
    bnshift = nc.dram_tensor("bnshift", [128, 2], F32, kind="ExternalInput")
    out = nc.dram_tensor("out", [C, N], F32, kind="ExternalOutput")

    with tile.TileContext(nc) as tc:
        with tc.tile_pool(name="const", bufs=1) as const, \
             tc.tile_pool(name="xpool", bufs=1) as xpool, \
             tc.tile_pool(name="proj", bufs=1) as proj, \
             tc.tile_pool(name="yt_pool", bufs=3) as yt_pool, \
             tc.tile_pool(name="small", bufs=6) as small, \
             tc.tile_pool(name="ostage", bufs=4) as ostage:

            # ---- constants ----
            wth_sb = const.tile([128, 2 * O], F16)
            wph_sb = const.tile([128, 2 * O], F16)
            wg_sb = const.tile([128, 2 * O], F16)
            for k in range(2):
                csl = slice(k * 128, (k + 1) * 128)
                osl = slice(k * O, (k + 1) * O)
                nc.sync.dma_start(wth_sb[:, osl], wth[csl, :])
                nc.sync.dma_start(wph_sb[:, osl], wph[csl, :])
                nc.sync.dma_start(wg_sb[:, osl], wg[csl, :])
            wout_sb = const.tile([O, C], BF16)
            nc.sync.dma_start(wout_sb[:], wout[:])
            bth_sb = const.tile([O, 1], F32)
            bph_sb = const.tile([O, 1], F32)
            bnscale_sb = const.tile([128, 2], F32)
            bnshift_sb = const.tile([128, 2], F32)
            nc.sync.dma_start(bth_sb[:], bth[:])
            nc.sync.dma_start(bph_sb[:], bph[:])
            nc.sync.dma_start(bnscale_sb[:], bnscale[:])
            nc.sync.dma_start(bnshift_sb[:], bnshift[:])

            # ---- x load (fp16, one copy serves everything) ----
            x_h = [xpool.tile([128, N], F16, name=f"x{i}_h") for i in range(2)]
            for i in range(2):
                rsl = slice(i * 128, (i + 1) * 128)
                for h in range(4):
                    hsl = slice(h * (N // 4), (h + 1) * (N // 4))
                    nc.sync.dma_start(x_h[i][:, hsl], xin[rsl, hsl])

            theta_h = proj.tile([O, N], F16)
            phi_h = proj.tile([O, N], F16)
            gt_sb = proj.tile([128, M_CHUNKS * (O + 1)], BF16)

            with tc.tile_pool(name="thph_ps", bufs=2, space="PSUM") as thph_ps, \
                 tc.tile_pool(name="gt_ps", bufs=2, space="PSUM") as gt_ps:
                # phi first (the S^T pass needs all of phi), then theta, then gt
                for (dst, w_sb, b_sb) in ((phi_h, wph_sb, bph_sb), (theta_h, wth_sb, bth_sb)):
                    for part in range(4):
                        pp = thph_ps.tile([128, 1024], F32)
                        for j in range(2):
                            cb = part * 1024 + j * 512
                            for k in range(2):
                                nc.tensor.matmul(
                                    pp[:, j * 512:(j + 1) * 512],
                                    w_sb[:, k * O:(k + 1) * O],
                                    x_h[k][:, cb:cb + 512],
                                    start=(k == 0), stop=(k == 1),
                                )
                        psl = slice(part * 1024, (part + 1) * 1024)
                        if part % 2 == 0:
                            nc.vector.tensor_scalar_add(dst[:, psl], pp[:], b_sb[:])
                        else:
                            nc.scalar.activation(
                                dst[:, psl], pp[:],
                                mybir.ActivationFunctionType.Identity, bias=b_sb[:],
                            )

                for c in range(M_CHUNKS):
                    gp = gt_ps.tile([128, O], F32)
                    for k in range(2):
                        nc.tensor.matmul(
                            gp[:],
                            x_h[k][:, c * 128:(c + 1) * 128],
                            wg_sb[:, k * O:(k + 1) * O],
                            start=(k == 0), stop=(k == 1),
                        )
                    base = c * (O + 1)
                    if c % 2 == 0:
                        nc.vector.tensor_copy(gt_sb[:, base:base + O], gp[:])
                    else:
                        nc.scalar.copy(gt_sb[:, base:base + O], gp[:])
                    nc.gpsimd.memset(gt_sb[:, base + O:base + O + 1], 1.0)

            # ---- software-pipelined attention ----
            with tc.tile_pool(name="pt_pool", bufs=16) as pt_pool, \
                 tc.tile_pool(name="st_ps", bufs=2, space="PSUM") as st_ps, \
                 tc.tile_pool(name="ya", bufs=2, space="PSUM") as ya:

                def mm2(y, pt, s, c0, c1, pt_c0):
                    # y accumulator slice gets chunks [c0, c1) of P^T tile pt.
                    # Two accumulators share one PSUM bank: only the first
                    # (s even) opens the group (start=True clears the whole
                    # bank); the second writes start=False onto cleared bits.
                    first = None
                    for c in range(c0, c1):
                        i = nc.tensor.matmul(
                            y[:],
                            pt[:, (c - pt_c0) * NT + s * 128:(c - pt_c0) * NT + (s + 1) * 128],
                            gt_sb[:, c * (O + 1):(c + 1) * (O + 1)],
                            start=(c == 0 and s % 2 == 0),
                            stop=(c == M_CHUNKS - 1),
                            skip_group_check=True,
                        )
                        if first is None:
                            first = i
                    return first

                def norm_transpose(y, yt_tile, col):
                    recip = small.tile([128, 1], F32)
                    nc.vector.reciprocal(recip[:], y[:, O:O + 1])
                    ynorm = small.tile([128, O], BF16)
                    nc.vector.tensor_scalar_mul(ynorm[:], y[:, 0:O], recip[:])
                    nc.sync.dma_start_transpose(yt_tile[:, col:col + 128], ynorm[:])

                pts_prev, yt_cur, yt_prev, yt_tail = None, None, None, None
                for it in range(N_TILES + 2):
                    nt = it if it < N_TILES else None
                    pv = it - 1 if 0 <= it - 1 < N_TILES else None
                    tl = it - 2 if 0 <= it - 2 < N_TILES else None
                    if nt is not None:
                        ntsl = slice(nt * NT, (nt + 1) * NT)
                        y01 = ya.tile([128, 260], F32, name="y01", tag="ya")
                        yt_cur = yt_pool.tile([O, NT], BF16)
                    if pv is not None:
                        y23 = ya.tile([128, 260], F32, name="y23", tag="ya")
                    pts_cur = []
                    n_grps = len(GROUPS)
                    for g in range(n_grps):
                        if nt is not None:
                            c0, c1 = GROUPS[g]
                            w = (c1 - c0) * NT
                            st = st_ps.tile([128, 3 * NT], F32, name="st", tag="st")
                            for c in range(c0, c1):
                                nc.tensor.matmul(
                                    st[:, (c - c0) * NT:(c - c0 + 1) * NT],
                                    phi_h[:, c * 128:(c + 1) * 128],
                                    theta_h[:, ntsl],
                                    start=True, stop=True,
                                )
                            pt = pt_pool.tile([128, 3 * NT], BF16)
                            nc.scalar.activation(pt[:, :w], st[:, :w], EXP)
                            pts_cur.append(pt)
                            if g > 0:
                                pc0, pc1 = GROUPS[g - 1]
                                ia = mm2(y01[:, 0:O + 1], pts_cur[g - 1], 0, pc0, pc1, pc0)
                                ib = mm2(y01[:, 130:259], pts_cur[g - 1], 1, pc0, pc1, pc0)
                                if pc0 == 0:
                                    add_dep_helper(ib.ins, ia.ins, sync=False,
                                                   reason="bank-pack: clear before first write")
                        if pv is not None:
                            c0, c1 = GROUPS[g]
                            ia = mm2(y23[:, 0:O + 1], pts_prev[g], 2, c0, c1, c0)
                            ib = mm2(y23[:, 130:259], pts_prev[g], 3, c0, c1, c0)
                            if c0 == 0:
                                add_dep_helper(ib.ins, ia.ins, sync=False,
                                               reason="bank-pack: clear before first write")
                        if g == 2 and tl is not None:
                            # output projection of tile tl (yt complete since
                            # the end of the previous iteration); borrows one
                            # S^T-ring slot for its PSUM
                            o01 = st_ps.tile([128, 1024], F32, name="o01", tag="st")
                            tlsl = slice(tl * NT, (tl + 1) * NT)
                            for ct in range(2):
                                osl = o01[:, ct * NT:(ct + 1) * NT]
                                nc.tensor.matmul(
                                    osl,
                                    wout_sb[:, ct * 128:(ct + 1) * 128],
                                    yt_tail[:],
                                    start=True, stop=True,
                                    skip_group_check=True,
                                )
                            for ct in range(2):
                                obn = ostage.tile([128, NT], F32)
                                nc.vector.tensor_scalar(
                                    obn[:], o01[:, ct * NT:(ct + 1) * NT],
                                    bnscale_sb[:, ct:ct + 1], bnshift_sb[:, ct:ct + 1],
                                    op0=MULT, op1=ADD,
                                )
                                ores = ostage.tile([128, NT], F32, name="ores")
                                nc.vector.tensor_tensor(
                                    ores[:], obn[:], x_h[ct][:, tlsl], op=ADD
                                )
                                nc.gpsimd.dma_start(out[ct * 128:(ct + 1) * 128, tlsl], ores[:])
                    if nt is not None:
                        c0, c1 = GROUPS[-1]
                        mm2(y01[:, 0:O + 1], pts_cur[-1], 0, c0, c1, c0)
                        mm2(y01[:, 130:259], pts_cur[-1], 1, c0, c1, c0)
                        norm_transpose(y01[:, 0:O + 1], yt_cur, 0)
                        norm_transpose(y01[:, 130:259], yt_cur, 128)
                    if pv is not None:
                        norm_transpose(y23[:, 0:O + 1], yt_prev, 256)
                        norm_transpose(y23[:, 130:259], yt_prev, 384)
                    pts_prev = pts_cur
                    yt_tail = yt_prev
                    yt_prev = yt_cur

    nc.finalize()
    return nc


_NC_CACHE = None


def _get_nc():
    global _NC_CACHE
    if _NC_CACHE is None:
        _NC_CACHE = build_nc()
    return _NC_CACHE


def _prepare_in_maps(inputs):
    x = np.ascontiguousarray(np.asarray(inputs["x"], dtype=np.float32)).reshape(B, C, N)
    xh = x.astype(np.float16)
    wth = np.ascontiguousarray(np.asarray(inputs["w_theta"], np.float32).T).astype(np.float16)
    wph = np.ascontiguousarray(np.asarray(inputs["w_phi"], np.float32).T).astype(np.float16)
    wg = np.ascontiguousarray(np.asarray(inputs["w_g"], np.float32).T).astype(np.float16)
    w_out = np.asarray(inputs["w_out"], np.float32)
    wout = np.ascontiguousarray(w_out.T).astype(ml_dtypes.bfloat16)
    bth = np.asarray(inputs["b_theta"], np.float32).reshape(O, 1)
    bph = np.asarray(inputs["b_phi"], np.float32).reshape(O, 1)
    inv = np.asarray(inputs["bn_gamma"], np.float32) / np.sqrt(
        np.asarray(inputs["bn_var"], np.float32) + BN_EPS)
    shift = (np.asarray(inputs["b_out"], np.float32) * inv
             + np.asarray(inputs["bn_beta"], np.float32)
             - np.asarray(inputs["bn_mean"], np.float32) * inv)
    # fold the g-branch bias through the output projection: softmax rows sum
    # to 1, so attn @ (g + b_g) = attn @ g + b_g, and w_out @ b_g is constant
    wob = wout.astype(np.float32) .T @ np.asarray(inputs["b_g"], np.float32)
    shift = shift + inv * wob
    bnscale = np.ascontiguousarray(inv.reshape(2, 128).T)
    bnshift = np.ascontiguousarray(shift.reshape(2, 128).T)

    shared = dict(wth=wth, wph=wph, wg=wg, wout=wout, bth=bth, bph=bph,
                  bnscale=bnscale, bnshift=bnshift)
    return [dict(shared, xin=np.ascontiguousarray(xh[b])) for b in range(B)]


def _install_ntff_shim():
    """This image's antenv lacks axon_hooks; provide it from trn_boot's
    ctypes implementation so trace=True can capture NTFF profiles."""
    import types
    try:
        import antenv.axon_hooks  # noqa: F401
        return
    except ImportError:
        pass
    if "/root/.axon_site" not in sys.path:
        sys.path.insert(0, "/root/.axon_site")
    from trn_agent_boot.trn_boot import _ntff_profile_via_ctypes
    hook = _ntff_profile_via_ctypes("/opt/axon/libaxon_pjrt.so")
    m = types.ModuleType("antenv.axon_hooks")
    m.get_axon_ntff_profile_hook = lambda: hook
    m.set_axon_ntff_profile_hook = lambda h: None
    sys.modules["antenv.axon_hooks"] = m


def run(inputs, trace=False):
    if trace:
        _install_ntff_shim()
    nc = _get_nc()
    in_maps = _prepare_in_maps(inputs)
    res = run_bass_kernel_spmd(nc, in_maps, list(range(B)), trace=trace)
    outs = np.stack([res.results[b]["out"] for b in range(B)])
    return outs.reshape(B, C, 64, 64), res


def kernel(**inputs) -> np.ndarray:
    out, _ = run(inputs)
    return out


if __name__ == "__main__":
    # quick CoreSim check of one core
    from concourse import bass_interp
    rng = np.random.default_rng(0)
    fake = {
        "x": rng.standard_normal((B, C, 64, 64)).astype(np.float32),
        "w_theta": (rng.standard_normal((O, C)) * 0.05).astype(np.float32),
        "b_theta": (rng.standard_normal(O) * 0.05).astype(np.float32),
        "w_phi": (rng.standard_normal((O, C)) * 0.05).astype(np.float32),
        "b_phi": (rng.standard_normal(O) * 0.05).astype(np.float32),
        "w_g": (rng.standard_normal((O, C)) * 0.05).astype(np.float32),
        "b_g": (rng.standard_normal(O) * 0.05).astype(np.float32),
        "w_out": (rng.standard_normal((C, O)) * 0.05).astype(np.float32),
        "b_out": (rng.standard_normal(C) * 0.05).astype(np.float32),
        "bn_gamma": rng.standard_normal(C).astype(np.float32),
        "bn_beta": rng.standard_normal(C).astype(np.float32),
        "bn_mean": rng.standard_normal(C).astype(np.float32),
        "bn_var": rng.uniform(0.5, 1.5, C).astype(np.float32),
    }
    nc = _get_nc()
    in_maps = _prepare_in_maps(fake)
    sim = bass_interp.CoreSim(nc)
    for k, v in in_maps[0].items():
        sim.tensor(k)[:] = v
    sim.simulate()
    got = np.asarray(sim.tensor("out"))

    x0 = fake["x"][0].reshape(C, N)
    th = fake["w_theta"] @ x0 + fake["b_theta"][:, None]
    ph = fake["w_phi"] @ x0 + fake["b_phi"][:, None]
    gg = fake["w_g"] @ x0 + fake["b_g"][:, None]
    s = th.T @ ph
    p = np.exp(s - s.max(1, keepdims=True))
    a = p / p.sum(1, keepdims=True)
    yy = a @ gg.T
    wy = fake["w_out"] @ yy.T + fake["b_out"][:, None]
    inv = fake["bn_gamma"] / np.sqrt(fake["bn_var"] + BN_EPS)
    bn = wy * inv[:, None] + (fake["bn_beta"] - fake["bn_mean"] * inv)[:, None]
    want = x0 + bn
    err = np.abs(got - want).max()
    print("CoreSim absmax err:", err, "rel:", err / np.abs(want).max())
